# revision 1
# baseline (speedup 1.0000x reference)
import sys

import numpy as np

if "/opt/trn_rl_repo" not in sys.path:
    sys.path.insert(0, "/opt/trn_rl_repo")

import bass_rust as _bass_rust
import concourse.bass as bass
import concourse.tile as tile
from concourse import mybir
from concourse.bass_utils import run_bass_kernel_spmd
from concourse.masks import make_identity

# Problem: x [4, 64, 64, 64] f32. xf = x.reshape(B,C,N), N=4096.
# scores = xf^T xf per batch; attn = softmax(scores, axis=-1);
# out = xf @ attn^T (per batch) reshaped + x.
#
# Sharding: 8 cores = (batch b = k//2) x (row-half = k%2); host permutes the
# j axis per core so its own i-rows come first (softmax over j is
# permutation-invariant) -> identical SPMD program, no collectives.
#
# Matmuls run in float32r (1 PE pass vs 4 for fp32) with an 11-bit
# split-compensation scheme: x = hi + lo, where hi=RTN11(x) and lo both
# round-trip exactly through the PE's ~12-bit operand rounding, so
# hi*hi + lo*hi + hi*lo is fp32-accurate; the dropped lo*lo term is ~2^-22.
# M1 packs (Ah+Al)*Bh into ONE K=128 matmul: lhsT=[Ah;Al], rhs=[Bh;Bh],
# plus a K=65 matmul [Ah;ones]*[Bl;-m] that also folds the softmax shift
# m_i = ||x_i||^2 (shift errors cancel exactly in the normalization).
# M2 = Xh^T E + Xl^T E; E's own PE rounding (~2^-13) is the scheme's only
# uncompensated term. The l_i row rides along as a ones-column in Xh.

B_, C, H, W = 4, 64, 64, 64
N = H * W          # 4096
NI = N // 2        # 2048 rows of i per core
NJT = N // 128     # 32 j-tiles
NIC = NI // 512    # 4 i-chunks of 512
FP = mybir.dt.float32
FPR = mybir.dt.float32r
SPLIT = 11

M2_SINGLE = False  # if True: single un-split M2 matmul (cheaper, less exact)


def build_nc(reps: int = 1) -> bass.Bass:
    nc = bass.Bass()

    a_stack_d = nc.dram_tensor("a_stack", [2 * C, N], FPR, kind="ExternalInput")
    a_aug_d = nc.dram_tensor("a_aug", [C + 1, N], FPR, kind="ExternalInput")
    b_hh_d = nc.dram_tensor("b_hh", [2 * C, NI], FPR, kind="ExternalInput")
    b_aug_d = nc.dram_tensor("b_aug", [C + 1, NI], FPR, kind="ExternalInput")
    # host-pretiled xft: [128, NJT*(C+1)], column block jt = j-tile jt's [128, 65]
    xh_d = nc.dram_tensor("xft_hi", [128, NJT * (C + 1)], FPR, kind="ExternalInput")
    xl_d = nc.dram_tensor("xft_lo", [128, NJT * (C + 1)], FPR, kind="ExternalInput")
    out_dram = nc.dram_tensor("outT", [NI, C], FP, kind="ExternalOutput")

    with tile.TileContext(nc) as tc:
        with (
            tc.tile_pool(name="const", bufs=1) as const,
            tc.tile_pool(name="epool", bufs=3) as epool,
            tc.tile_pool(name="osb", bufs=2) as osb_pool,
            tc.tile_pool(name="small", bufs=3) as small,
            tc.tile_pool(name="res", bufs=3) as res_pool,
            tc.tile_pool(name="ps_s", bufs=4, space="PSUM") as ps_s,
            tc.tile_pool(name="ps_o", bufs=2, space="PSUM") as ps_o,
            tc.tile_pool(name="ps_t", bufs=2, space="PSUM") as ps_t,
        ):
            a_stack = const.tile([2 * C, N], FPR)
            a_aug = const.tile([C + 1, N], FPR)
            b_hh = const.tile([2 * C, NI], FPR)
            b_aug = const.tile([C + 1, NI], FPR)
            xh_sb = const.tile([128, NJT, C + 1], FPR)
            xl_sb = const.tile([128, NJT, C + 1], FPR)
            identity = const.tile([128, 128], FP)

            make_identity(nc, identity[:])

            # Input DMAs: early-needed chunks first. CRITICAL: keep the ACT
            # (nc.scalar) queue empty — DMA descriptors occupy ~630ns of queue
            # issue each and would block exp() behind them for ~20us.
            sl0 = slice(0, 512)
            j0 = slice(0, 128)
            j0r = slice(128, 512)
            nc.sync.dma_start(out=b_hh[:, sl0], in_=b_hh_d[:, sl0])
            nc.sync.dma_start(out=a_stack[:, j0], in_=a_stack_d[:, j0])
            nc.sync.dma_start(out=b_aug[:, sl0], in_=b_aug_d[:, sl0])
            nc.sync.dma_start(out=a_aug[:, j0], in_=a_aug_d[:, j0])
            nc.sync.dma_start(out=a_stack[:, j0r], in_=a_stack_d[:, j0r])
            nc.sync.dma_start(out=a_aug[:, j0r], in_=a_aug_d[:, j0r])
            for cidx in range(1, 8):
                sl = slice(cidx * 512, (cidx + 1) * 512)
                nc.sync.dma_start(out=a_stack[:, sl], in_=a_stack_d[:, sl])
                nc.sync.dma_start(out=a_aug[:, sl], in_=a_aug_d[:, sl])
            for cidx in range(1, 4):
                sl = slice(cidx * 512, (cidx + 1) * 512)
                nc.sync.dma_start(out=b_hh[:, sl], in_=b_hh_d[:, sl])
                nc.sync.dma_start(out=b_aug[:, sl], in_=b_aug_d[:, sl])
            for t in range(4):
                jt0, jt1 = t * 8, (t + 1) * 8
                sl = slice(jt0 * (C + 1), jt1 * (C + 1))
                nc.gpsimd.dma_start(out=xh_sb[:, jt0:jt1, :], in_=xh_d[:, sl])
                nc.gpsimd.dma_start(out=xl_sb[:, jt0:jt1, :], in_=xl_d[:, sl])

            for _rep in range(reps):
                # Per 512-wide i-chunk, sweep 32 j-tiles:
                #   M1a: [Ah;Al]^T [Bh;Bh] -> psum_s  (hi*hi + lo*hi)
                #   M1b: [Ah;1]^T [Bl;-m]  += psum_s  (hi*lo - m_i)
                #   E = exp(psum_s)
                #   M2:  Xh^T E + Xl^T E   -> psum_o [65, 512]
                # Row 64 of psum_o accumulates l_i = sum_j E.
                for ic in range(NIC):
                    isl = slice(ic * 512, (ic + 1) * 512)
                    psum_o = ps_o.tile([C + 1, 512], FP)

                    def emit_m2(jt, e):
                        if M2_SINGLE:
                            nc.tensor.matmul(
                                psum_o[:], xh_sb[:, jt, :], e[:],
                                start=(jt == 0), stop=(jt == NJT - 1),
                            )
                        else:
                            nc.tensor.matmul(
                                psum_o[:], xh_sb[:, jt, :], e[:],
                                start=(jt == 0), stop=False,
                            )
                            nc.tensor.matmul(
                                psum_o[:], xl_sb[:, jt, :], e[:],
                                start=False, stop=(jt == NJT - 1),
                            )

                    # M2 is emitted one j-tile behind M1 so the PE queue never
                    # blocks on exp(jt): while ACT runs exp(jt), PE does M1(jt+1).
                    e_prev = None
                    for jt in range(NJT):
                        jsl = slice(jt * 128, (jt + 1) * 128)
                        psum_s = ps_s.tile([128, 512], FP)
                        nc.tensor.matmul(
                            psum_s[:], a_stack[:, jsl], b_hh[:, isl],
                            start=True, stop=False,
                        )
                        nc.tensor.matmul(
                            psum_s[:], a_aug[:, jsl], b_aug[:, isl],
                            start=False, stop=True,
                        )
                        e_sb = epool.tile([128, 512], FPR)
                        nc.scalar.activation(
                            e_sb[:], psum_s[:], mybir.ActivationFunctionType.Exp
                        )
                        if e_prev is not None:
                            emit_m2(jt - 1, e_prev)
                        e_prev = e_sb
                    emit_m2(NJT - 1, e_prev)

                    # Epilogue: transpose 4 pieces of [65, 128] -> [128, 65],
                    # normalize by row 64, add residual (= Xh + Xl), DMA out.
                    o_sb = osb_pool.tile([C + 1, 512], FP)
                    nc.vector.tensor_copy(o_sb[:], psum_o[:])
                    resb = res_pool.tile([128, 4, C], FP)
                    for t in range(4):
                        piece = ic * 4 + t
                        tr_psum = ps_t.tile([128, C + 1], FP)
                        nc.tensor.transpose(
                            tr_psum[:],
                            o_sb[:, t * 128 : (t + 1) * 128],
                            identity[0 : C + 1, 0 : C + 1],
                        )
                        rl = small.tile([128, 1], FP)
                        nc.vector.reciprocal(rl[:], tr_psum[:, C : C + 1])
                        nc.vector.tensor_scalar_mul(
                            resb[:, t, :], tr_psum[:, 0:C], rl[:]
                        )
                        nc.vector.tensor_add(
                            resb[:, t, :], resb[:, t, :],
                            xh_sb[:, piece, 0:C].bitcast(FP),
                        )
                        nc.vector.tensor_add(
                            resb[:, t, :], resb[:, t, :],
                            xl_sb[:, piece, 0:C].bitcast(FP),
                        )
                    nc.sync.dma_start(
                        out=out_dram[ic * 512 : (ic + 1) * 512, :].rearrange(
                            "(t p) c -> p t c", t=4
                        ),
                        in_=resb[:],
                    )

    # TRN2 allows at most 1 sync wait per instruction (2 on EventSemaphore);
    # split excess waits like Bacc.compile does, else walrus codegen fails.
    _bass_rust.generate_event_semaphores(nc)
    return nc


def _round_rtn(x: np.ndarray, k: int) -> np.ndarray:
    u = np.ascontiguousarray(x, dtype=np.float32).view(np.uint32).astype(np.uint64)
    half = np.uint64(1 << (24 - k - 1))
    mask = np.uint64(0xFFFFFFFF) << np.uint64(24 - k)
    return ((u + half) & mask).astype(np.uint32).view(np.float32)


def prepare_in_maps(x: np.ndarray) -> list[dict[str, np.ndarray]]:
    xf_full = np.asarray(x, dtype=np.float32).reshape(B_, C, N)
    in_maps = []
    for k in range(8):
        b, half = k // 2, k % 2
        xf_b = xf_full[b]
        if half == 0:
            xf_core = xf_b
        else:
            xf_core = np.concatenate([xf_b[:, NI:], xf_b[:, :NI]], axis=1)
        xf_core = np.ascontiguousarray(xf_core)

        hi = _round_rtn(xf_core, SPLIT)
        lo = (xf_core - hi).astype(np.float32)
        m32 = (
            (xf_core[:, :NI].astype(np.float64) ** 2).sum(axis=0)
        ).astype(np.float32)

        a_stack = np.concatenate([hi, lo], axis=0)
        a_aug = np.concatenate([hi, np.ones((1, N), np.float32)], axis=0)
        b_hh = np.concatenate([hi[:, :NI], hi[:, :NI]], axis=0)
        b_aug = np.concatenate([lo[:, :NI], -m32[None, :]], axis=0)
        ones_col = np.ones((N, 1), np.float32)
        zeros_col = np.zeros((N, 1), np.float32)
        # pretile [N, 65] -> [128, NJT*65] so SBUF loads are contiguous chunks
        xft_hi = (
            np.concatenate([hi.T, ones_col], axis=1)
            .reshape(NJT, 128, C + 1)
            .transpose(1, 0, 2)
            .reshape(128, NJT * (C + 1))
        )
        xft_lo = (
            np.concatenate([lo.T, zeros_col], axis=1)
            .reshape(NJT, 128, C + 1)
            .transpose(1, 0, 2)
            .reshape(128, NJT * (C + 1))
        )
        in_maps.append(
            {
                "a_stack": np.ascontiguousarray(a_stack),
                "a_aug": np.ascontiguousarray(a_aug),
                "b_hh": np.ascontiguousarray(b_hh),
                "b_aug": np.ascontiguousarray(b_aug),
                "xft_hi": np.ascontiguousarray(xft_hi),
                "xft_lo": np.ascontiguousarray(xft_lo),
            }
        )
    return in_maps


def gather_output(results: list[dict[str, np.ndarray]]) -> np.ndarray:
    out_full = np.empty((B_, C, N), dtype=np.float32)
    for k in range(8):
        b, half = k // 2, k % 2
        i0 = half * NI
        outT = results[k]["outT"]  # [2048, 64] rows=i, cols=c
        out_full[b][:, i0 : i0 + NI] = outT.T
    return out_full.reshape(B_, C, H, W)


def kernel_run(x: np.ndarray, trace: bool = False):
    nc = build_nc()
    in_maps = prepare_in_maps(x)
    r = run_bass_kernel_spmd(nc, in_maps, list(range(8)), trace=trace)
    out = gather_output(r.results)
    return out, (r.exec_time_ns if trace else None)


def kernel(**inputs: np.ndarray) -> np.ndarray:
    out, _ = kernel_run(inputs["x"], trace=False)
    return out



# revision 10
# speedup vs baseline: 1.2083x; 1.2083x over previous
import sys

import numpy as np

if "/opt/trn_rl_repo" not in sys.path:
    sys.path.insert(0, "/opt/trn_rl_repo")

import bass_rust as _bass_rust
import concourse.bass as bass
import concourse.tile as tile
from concourse import mybir
from concourse.bass_utils import run_bass_kernel_spmd
from concourse.masks import make_identity

# Problem: x [4, 64, 64, 64] f32. xf = x.reshape(B,C,N), N=4096.
# scores = xf^T xf per batch; attn = softmax(scores, axis=-1);
# out = xf @ attn^T (per batch) reshaped + x.
#
# Sharding: 8 cores = (batch b = k//2) x (row-half = k%2); host permutes the
# j axis per core so its own i-rows come first (softmax over j is
# permutation-invariant) -> identical SPMD program, no collectives.
# With that permutation the diagonal (j == i) always lands in j-tile
# jt = i//128, i.e. pairs {2*ic, 2*ic+1} of each 512-wide i-chunk.
#
# Math per core: single-pass float32r matmuls (PE operand rounding ~2^-12
# gives ~5e-5 final rel err, far inside the 2e-2 gate; no hi/lo split).
# M1: psum[j,i] = [x;1]^T [x; K-m] = s_ji - m_i + K   (K-m rides as row 64;
#     the shift rounding error is constant per softmax row and cancels).
# exp: 24/32 j-tiles per i-chunk on ACT (exact, bias=-K), 8 provably
#     off-diagonal tiles on DVE via Schraudolph exp2 bit trick
#     (uint32((psum max 0) * 2^23/ln2) bitcast as float ~ e^(psum-K),
#     max rel err ~3% on entries that are all ~<=1e-4 of the row mass).
#     ACT alone (1.2 GHz) cannot keep up with the PE (2.4 GHz).
# M2: psum_o[c,i] += xft[jt]^T E, K=128; ones column accumulates l_i.
# Epilogue: PE-transpose [65,128] pieces, normalize by l, add residual x
# (xft rows are exactly x^T), DMA out.

B_, C, H, W = 4, 64, 64, 64
N = H * W          # 4096
NI = N // 2        # 2048 rows of i per core
NJT = N // 128     # 32 j-tiles
NIC = NI // 512    # 4 i-chunks of 512
NP = NJT // 2      # 16 j-tile pairs
FP = mybir.dt.float32
FPR = mybir.dt.float32r
BF16 = mybir.dt.bfloat16
U16 = mybir.dt.uint16

LN2 = 0.6931471805599453
K_SHIFT = (127.0 - 0.0434609) * LN2   # Schraudolph bias, ~88.0
C1B = float(2**7) / LN2               # Schraudolph scale for bf16 bits


def dve_pairs(ic: int) -> set[int]:
    # 4 of 16 pairs per i-chunk go to DVE; never the diagonal pairs
    # {2*ic, 2*ic+1}, spread every 4th so ACT/DVE interleave evenly.
    return {(2 * ic + 2 + 4 * k) % NP for k in range(4)}


def build_nc(reps: int = 1) -> bass.Bass:
    nc = bass.Bass()

    a_aug_d = nc.dram_tensor("a_aug", [C + 1, N], FPR, kind="ExternalInput")
    b_aug_d = nc.dram_tensor("b_aug", [C + 1, NI], FPR, kind="ExternalInput")
    # host-pretiled x^T: [128, NJT*(C+1)], block jt = j-tile jt's [128, 65]
    xft_d = nc.dram_tensor("xft", [128, NJT * (C + 1)], FPR, kind="ExternalInput")
    # bf16 copy for the Schraudolph-pair M2 matmuls (verifier requires
    # fp32r matmul operands to be producer-rounded; bf16 has no such rule)
    xfb_d = nc.dram_tensor("xfb", [128, NJT * (C + 1)], BF16, kind="ExternalInput")
    out_dram = nc.dram_tensor("outT", [NI, C], FP, kind="ExternalOutput")

    with tile.TileContext(nc) as tc:
        with (
            tc.tile_pool(name="const", bufs=1) as const,
            tc.tile_pool(name="epool", bufs=3) as epool,
            tc.tile_pool(name="u32p", bufs=3) as u32p,
            tc.tile_pool(name="osb", bufs=2) as osb_pool,
            tc.tile_pool(name="small", bufs=3) as small,
            tc.tile_pool(name="res", bufs=2) as res_pool,
            tc.tile_pool(name="ps_s", bufs=2, space="PSUM") as ps_s,
            tc.tile_pool(name="ps_o", bufs=2, space="PSUM") as ps_o,
            tc.tile_pool(name="ps_t", bufs=2, space="PSUM") as ps_t,
        ):
            a_aug = const.tile([C + 1, N], FPR)
            b_aug = const.tile([C + 1, NI], FPR)
            xft = const.tile([128, NJT, C + 1], FPR)
            xfb = const.tile([128, NJT, C + 1], BF16)
            identity = const.tile([128, 128], FP)
            nbias = const.tile([128, 1], FP)

            make_identity(nc, identity[:])
            nc.gpsimd.memset(nbias[:], -K_SHIFT)

            # Input DMAs: early-needed chunks first; keep the ACT queue free
            # of DMA descriptors (they'd block exp issue).
            nc.sync.dma_start(out=b_aug[:, 0:512], in_=b_aug_d[:, 0:512])
            nc.sync.dma_start(out=a_aug[:, 0:256], in_=a_aug_d[:, 0:256])
            nc.sync.dma_start(out=a_aug[:, 256:512], in_=a_aug_d[:, 256:512])
            for cidx in range(1, 8):
                sl = slice(cidx * 512, (cidx + 1) * 512)
                nc.sync.dma_start(out=a_aug[:, sl], in_=a_aug_d[:, sl])
            for cidx in range(1, 4):
                sl = slice(cidx * 512, (cidx + 1) * 512)
                nc.sync.dma_start(out=b_aug[:, sl], in_=b_aug_d[:, sl])
            for t in range(4):
                jt0, jt1 = t * 8, (t + 1) * 8
                sl = slice(jt0 * (C + 1), jt1 * (C + 1))
                nc.gpsimd.dma_start(out=xft[:, jt0:jt1, :], in_=xft_d[:, sl])
                nc.gpsimd.dma_start(out=xfb[:, jt0:jt1, :], in_=xfb_d[:, sl])

            for _rep in range(reps):
                for ic in range(NIC):
                    isl = slice(ic * 512, (ic + 1) * 512)
                    psum_o = ps_o.tile([C + 1, 512], FP)
                    dset = dve_pairs(ic)

                    def emit_m2(p, w, e0, e1):
                        nc.tensor.matmul(
                            psum_o[:], w[:, 2 * p, :], e0,
                            start=(p == 0), stop=False,
                        )
                        nc.tensor.matmul(
                            psum_o[:], w[:, 2 * p + 1, :], e1,
                            start=False, stop=(p == NP - 1),
                        )

                    # M2 emitted one pair behind M1/exp so the PE never
                    # waits on exp(p) while M1(p+1) is available.
                    prev = None
                    for p in range(NP):
                        j0 = slice(2 * p * 128, (2 * p + 1) * 128)
                        j1 = slice((2 * p + 1) * 128, (2 * p + 2) * 128)
                        psum_s = ps_s.tile([128, 1024], FP)
                        nc.tensor.matmul(
                            psum_s[:, 0:512], a_aug[:, j0], b_aug[:, isl],
                            start=True, stop=True,
                        )
                        nc.tensor.matmul(
                            psum_s[:, 512:1024], a_aug[:, j1], b_aug[:, isl],
                            start=True, stop=True,
                        )
                        if p in dset:
                            e_u = u32p.tile([128, 1024], U16)
                            nc.vector.tensor_scalar(
                                e_u[:], psum_s[:], 0.0, C1B,
                                mybir.AluOpType.max, mybir.AluOpType.mult,
                            )
                            e01 = (
                                xfb,
                                e_u[:, 0:512].bitcast(BF16),
                                e_u[:, 512:1024].bitcast(BF16),
                            )
                        else:
                            e_sb = epool.tile([128, 1024], FPR)
                            nc.scalar.activation(
                                e_sb[:], psum_s[:],
                                mybir.ActivationFunctionType.Exp,
                                bias=nbias[:], scale=1.0,
                            )
                            e01 = (xft, e_sb[:, 0:512], e_sb[:, 512:1024])
                        if prev is not None:
                            emit_m2(*prev)
                        prev = (p, *e01)
                    emit_m2(*prev)

                    # Epilogue: transpose 4 pieces of [65, 128] -> [128, 65],
                    # normalize by row 64 (= l_i), add residual x, DMA out.
                    o_sb = osb_pool.tile([C + 1, 512], FP)
                    nc.vector.tensor_copy(o_sb[:], psum_o[:])
                    resb = res_pool.tile([128, 4, C], FP)
                    for t in range(4):
                        piece = ic * 4 + t
                        tr_psum = ps_t.tile([128, C + 1], FP)
                        nc.tensor.transpose(
                            tr_psum[:],
                            o_sb[:, t * 128 : (t + 1) * 128],
                            identity[0 : C + 1, 0 : C + 1],
                        )
                        rl = small.tile([128, 1], FP)
                        nc.vector.reciprocal(rl[:], tr_psum[:, C : C + 1])
                        nc.vector.tensor_scalar_mul(
                            resb[:, t, :], tr_psum[:, 0:C], rl[:]
                        )
                        nc.vector.tensor_add(
                            resb[:, t, :], resb[:, t, :],
                            xft[:, piece, 0:C].bitcast(FP),
                        )
                    nc.sync.dma_start(
                        out=out_dram[ic * 512 : (ic + 1) * 512, :].rearrange(
                            "(t p) c -> p t c", t=4
                        ),
                        in_=resb[:],
                    )

    # TRN2 allows at most 1 sync wait per instruction (2 on EventSemaphore);
    # split excess waits like Bacc.compile does, else walrus codegen fails.
    _bass_rust.generate_event_semaphores(nc)
    return nc


def prepare_in_maps(x: np.ndarray) -> list[dict[str, np.ndarray]]:
    xf_full = np.asarray(x, dtype=np.float32).reshape(B_, C, N)
    ones_row = np.ones((1, N), np.float32)
    in_maps = []
    for k in range(8):
        b, half = k // 2, k % 2
        xf_b = xf_full[b]
        if half == 0:
            xf_core = xf_b
        else:
            xf_core = np.concatenate([xf_b[:, NI:], xf_b[:, :NI]], axis=1)
        xf_core = np.ascontiguousarray(xf_core)

        m64 = (xf_core[:, :NI].astype(np.float64) ** 2).sum(axis=0)
        shift_row = (K_SHIFT - m64).astype(np.float32)

        a_aug = np.concatenate([xf_core, ones_row], axis=0)
        b_aug = np.concatenate([xf_core[:, :NI], shift_row[None, :]], axis=0)
        # pretile [N, 65] -> [128, NJT*65] so SBUF loads are contiguous chunks
        xft = (
            np.concatenate([xf_core.T, np.ones((N, 1), np.float32)], axis=1)
            .reshape(NJT, 128, C + 1)
            .transpose(1, 0, 2)
            .reshape(128, NJT * (C + 1))
        )
        import ml_dtypes

        xfb = xft.astype(ml_dtypes.bfloat16)
        in_maps.append(
            {
                "a_aug": np.ascontiguousarray(a_aug),
                "b_aug": np.ascontiguousarray(b_aug),
                "xft": np.ascontiguousarray(xft),
                "xfb": np.ascontiguousarray(xfb),
            }
        )
    return in_maps


def gather_output(results: list[dict[str, np.ndarray]]) -> np.ndarray:
    out_full = np.empty((B_, C, N), dtype=np.float32)
    for k in range(8):
        b, half = k // 2, k % 2
        i0 = half * NI
        outT = results[k]["outT"]  # [2048, 64] rows=i, cols=c
        out_full[b][:, i0 : i0 + NI] = outT.T
    return out_full.reshape(B_, C, H, W)


def kernel_run(x: np.ndarray, trace: bool = False):
    nc = build_nc()
    in_maps = prepare_in_maps(x)
    r = run_bass_kernel_spmd(nc, in_maps, list(range(8)), trace=trace)
    out = gather_output(r.results)
    return out, (r.exec_time_ns if trace else None)


def kernel(**inputs: np.ndarray) -> np.ndarray:
    out, _ = kernel_run(inputs["x"], trace=False)
    return out


# revision 20
# speedup vs baseline: 1.7012x; 1.4079x over previous
import sys

import numpy as np

if "/opt/trn_rl_repo" not in sys.path:
    sys.path.insert(0, "/opt/trn_rl_repo")

import bass_rust as _bass_rust
import concourse.bass as bass
import concourse.tile as tile
from concourse import mybir
from concourse.bass_utils import run_bass_kernel_spmd
from concourse.masks import make_identity

# Problem: x [4, 64, 64, 64] f32. xf = x.reshape(B,C,N), N=4096.
# scores = xf^T xf per batch; attn = softmax(scores, axis=-1);
# out = xf @ attn^T (per batch) reshaped + x.
#
# Sharding: 8 cores = (batch b = k//2) x (row-half = k%2); host permutes the
# j axis per core so its own i-rows come first (softmax over j is
# permutation-invariant) -> identical SPMD program, no collectives.
# With that permutation the diagonal (j == i) always lands in j-tile
# jt = i//128, i.e. pairs {2*ic, 2*ic+1} of each 512-wide i-chunk.
#
# Math per core: single-pass float32r matmuls (PE operand rounding ~2^-12
# gives ~5e-5 final rel err, far inside the 2e-2 gate; no hi/lo split).
# M1: psum[j,i] = [x;1]^T [x; K-m] = s_ji - m_i + K   (K-m rides as row 64;
#     the shift rounding error is constant per softmax row and cancels).
# exp: 18/32 j-tiles per i-chunk on ACT (exact exp, bias=-K); 14 provably
#     off-diagonal tiles on DVE via a Schraudolph exp2 bit trick:
#     uint16((psum max 0) * 2^7/ln2) bitcast as bf16 ~ e^(psum-K), max rel
#     err ~3%. (bf16 bits, not fp32: the BIR verifier requires fp32r matmul
#     operands to be producer-rounded, and bf16 covers the full e^+32
#     dynamic range the data's hot off-diagonal scores actually reach.)
#     ACT alone cannot keep up with the PE; DVE takes the overflow.
# M2: psum_o[c,i] += xft[jt]^T E, K=128 (bf16 weights for DVE tiles);
#     ones column accumulates l_i.
# Epilogue: PE-transpose [65,128] pieces, normalize by l, add residual x
# (xft rows are exactly x^T), DMA out.
#
# Perf notes (measured): exp latency/pair ~1.24us > PE work/pair 854ns, so
# M2 runs TWO pairs behind M1 (psum_s bufs=3); one pair behind measured
# ~90us, two-behind ~57us = the PE stream roofline (131072 moving columns
# at 2.4GHz = 54.6us, + transposes/decode). ACT (39us) and DVE (39us) sit
# just below the PE. Dead ends tried:
# - fp8 DoubleRow M2: e5m2 lacks range for the hot off-diag scores -> NaN;
#   LDWEIGHTS overhead made it slower anyway (59us).
# - operand-swapped M2 (E stationary): 8x[128,128] weight loads per pair
#   don't hide behind 65-col streams -> slower (75us).
# - PE row-tiling M1 as 2 concurrent K=64 matmuls (tile_position (0,0)/
#   (64,0)): structurally impossible. This data's scores span ~5..202
#   (diag max 202!), a ~197 e-fold range; fp32's e^{-103}..e^{+88} window
#   cannot hold exp(s - c) for any CONSTANT c, so the per-row m_i shift is
#   mandatory -> K=65 -> no room in the 32-row-quantized tile groups. No
#   other engine can add a per-COLUMN shift to PSUM (ACT bias is
#   per-partition, GPSIMD can't read PSUM, DVE has no partition-broadcast).
# - M2 col-tiling (2 j-tiles/512 cycles) blocked by the l ones-column
#   making M=65 > 64; no alternative partition-reducer exists for l.

B_, C, H, W = 4, 64, 64, 64
N = H * W          # 4096
NI = N // 2        # 2048 rows of i per core
NJT = N // 128     # 32 j-tiles
NIC = NI // 512    # 4 i-chunks of 512
NP = NJT // 2      # 16 j-tile pairs
FP = mybir.dt.float32
FPR = mybir.dt.float32r
BF16 = mybir.dt.bfloat16
U16 = mybir.dt.uint16

LN2 = 0.6931471805599453
K_SHIFT = (127.0 - 0.0434609) * LN2   # Schraudolph bias, ~88.0
C1B = float(2**7) / LN2               # Schraudolph scale for bf16 bits


def dve_pairs(ic: int) -> set[int]:
    # 7 of 16 pairs per i-chunk go to DVE; never the diagonal pairs
    # {2*ic, 2*ic+1}, stepping by 2 so ACT/DVE interleave evenly.
    return {(2 * ic + 2 + 2 * k) % NP for k in range(7)}


def build_nc(reps: int = 1) -> bass.Bass:
    nc = bass.Bass()

    a_aug_d = nc.dram_tensor("a_aug", [C + 1, N], FPR, kind="ExternalInput")
    b_aug_d = nc.dram_tensor("b_aug", [C + 1, NI], FPR, kind="ExternalInput")
    # host-pretiled x^T: [128, NJT*(C+1)], block jt = j-tile jt's [128, 65]
    xft_d = nc.dram_tensor("xft", [128, NJT * (C + 1)], FPR, kind="ExternalInput")
    # bf16 copy for the Schraudolph-pair M2 matmuls (verifier requires
    # fp32r matmul operands to be producer-rounded; bf16 has no such rule)
    xfb_d = nc.dram_tensor("xfb", [128, NJT * (C + 1)], BF16, kind="ExternalInput")
    out_dram = nc.dram_tensor("outT", [NI, C], FP, kind="ExternalOutput")

    with tile.TileContext(nc) as tc:
        with (
            tc.tile_pool(name="const", bufs=1) as const,
            tc.tile_pool(name="epool", bufs=3) as epool,
            tc.tile_pool(name="u32p", bufs=3) as u32p,
            tc.tile_pool(name="osb", bufs=2) as osb_pool,
            tc.tile_pool(name="small", bufs=3) as small,
            tc.tile_pool(name="res", bufs=2) as res_pool,
            tc.tile_pool(name="ps_s", bufs=3, space="PSUM") as ps_s,
            tc.tile_pool(name="ps_o", bufs=2, space="PSUM") as ps_o,
        ):
            a_aug = const.tile([C + 1, N], FPR)
            b_aug = const.tile([C + 1, NI], FPR)
            xft = const.tile([128, NJT, C + 1], FPR)
            xfb = const.tile([128, NJT, C + 1], BF16)
            identity = const.tile([128, 128], FP)
            nbias = const.tile([128, 1], FP)

            make_identity(nc, identity[:])
            nc.gpsimd.memset(nbias[:], -K_SHIFT)

            # Input DMAs: early-needed chunks first; keep the ACT queue free
            # of DMA descriptors (they'd block exp issue).
            nc.sync.dma_start(out=b_aug[:, 0:512], in_=b_aug_d[:, 0:512])
            nc.sync.dma_start(out=a_aug[:, 0:256], in_=a_aug_d[:, 0:256])
            nc.sync.dma_start(out=a_aug[:, 256:512], in_=a_aug_d[:, 256:512])
            for cidx in range(1, 8):
                sl = slice(cidx * 512, (cidx + 1) * 512)
                nc.sync.dma_start(out=a_aug[:, sl], in_=a_aug_d[:, sl])
            for cidx in range(1, 4):
                sl = slice(cidx * 512, (cidx + 1) * 512)
                nc.sync.dma_start(out=b_aug[:, sl], in_=b_aug_d[:, sl])
            for t in range(4):
                jt0, jt1 = t * 8, (t + 1) * 8
                sl = slice(jt0 * (C + 1), jt1 * (C + 1))
                nc.gpsimd.dma_start(out=xft[:, jt0:jt1, :], in_=xft_d[:, sl])
                nc.gpsimd.dma_start(out=xfb[:, jt0:jt1, :], in_=xfb_d[:, sl])

            for _rep in range(reps):
                for ic in range(NIC):
                    isl = slice(ic * 512, (ic + 1) * 512)
                    psum_o = ps_o.tile([C + 1, 512], FP, tag="ot")
                    dset = dve_pairs(ic)

                    def emit_m2(p, w, e0, e1):
                        nc.tensor.matmul(
                            psum_o[:], w[:, 2 * p, :], e0,
                            start=(p == 0), stop=False,
                        )
                        nc.tensor.matmul(
                            psum_o[:], w[:, 2 * p + 1, :], e1,
                            start=False, stop=(p == NP - 1),
                        )

                    # M2 emitted TWO pairs behind M1/exp: exp latency
                    # (~1.24us/pair incl. semaphores) exceeds PE work per
                    # pair (~854ns), so one-behind stalls the PE on every
                    # pair (measured: 87us vs 60us predicted). Two-behind
                    # gives exp 2 slots of slack; needs psum_s bufs=3.
                    pend = []
                    for p in range(NP):
                        j0 = slice(2 * p * 128, (2 * p + 1) * 128)
                        j1 = slice((2 * p + 1) * 128, (2 * p + 2) * 128)
                        psum_s = ps_s.tile([128, 1024], FP)
                        nc.tensor.matmul(
                            psum_s[:, 0:512], a_aug[:, j0], b_aug[:, isl],
                            start=True, stop=True,
                        )
                        nc.tensor.matmul(
                            psum_s[:, 512:1024], a_aug[:, j1], b_aug[:, isl],
                            start=True, stop=True,
                        )
                        if p in dset:
                            e_u = u32p.tile([128, 1024], U16)
                            nc.vector.tensor_scalar(
                                e_u[:], psum_s[:], 0.0, C1B,
                                mybir.AluOpType.max, mybir.AluOpType.mult,
                            )
                            e01 = (
                                xfb,
                                e_u[:, 0:512].bitcast(BF16),
                                e_u[:, 512:1024].bitcast(BF16),
                            )
                        else:
                            e_sb = epool.tile([128, 1024], FPR)
                            nc.scalar.activation(
                                e_sb[:], psum_s[:],
                                mybir.ActivationFunctionType.Exp,
                                bias=nbias[:], scale=1.0,
                            )
                            e01 = (xft, e_sb[:, 0:512], e_sb[:, 512:1024])
                        pend.append((p, *e01))
                        if len(pend) > 2:
                            emit_m2(*pend.pop(0))
                    for item in pend:
                        emit_m2(*item)

                    # Epilogue: transpose 4 pieces of [65, 128] -> [128, 65],
                    # normalize by row 64 (= l_i), add residual x, DMA out.
                    o_sb = osb_pool.tile([C + 1, 512], FP)
                    nc.vector.tensor_copy(o_sb[:], psum_o[:])
                    resb = res_pool.tile([128, 4, C], FP)
                    for t in range(4):
                        piece = ic * 4 + t
                        # transpose tiles share the ps_o pool slots (PSUM
                        # budget: 3x2 banks ps_s + 2 banks ps_o = 8)
                        tr_psum = ps_o.tile([128, C + 1], FP, tag="ot")
                        nc.tensor.transpose(
                            tr_psum[:],
                            o_sb[:, t * 128 : (t + 1) * 128],
                            identity[0 : C + 1, 0 : C + 1],
                        )
                        rl = small.tile([128, 1], FP)
                        nc.vector.reciprocal(rl[:], tr_psum[:, C : C + 1])
                        nc.vector.tensor_scalar_mul(
                            resb[:, t, :], tr_psum[:, 0:C], rl[:]
                        )
                        nc.vector.tensor_add(
                            resb[:, t, :], resb[:, t, :],
                            xft[:, piece, 0:C].bitcast(FP),
                        )
                    nc.sync.dma_start(
                        out=out_dram[ic * 512 : (ic + 1) * 512, :].rearrange(
                            "(t p) c -> p t c", t=4
                        ),
                        in_=resb[:],
                    )

    # TRN2 allows at most 1 sync wait per instruction (2 on EventSemaphore);
    # split excess waits like Bacc.compile does, else walrus codegen fails.
    _bass_rust.generate_event_semaphores(nc)
    return nc


def prepare_in_maps(x: np.ndarray) -> list[dict[str, np.ndarray]]:
    xf_full = np.asarray(x, dtype=np.float32).reshape(B_, C, N)
    ones_row = np.ones((1, N), np.float32)
    in_maps = []
    for k in range(8):
        b, half = k // 2, k % 2
        xf_b = xf_full[b]
        if half == 0:
            xf_core = xf_b
        else:
            xf_core = np.concatenate([xf_b[:, NI:], xf_b[:, :NI]], axis=1)
        xf_core = np.ascontiguousarray(xf_core)

        m64 = (xf_core[:, :NI].astype(np.float64) ** 2).sum(axis=0)
        shift_row = (K_SHIFT - m64).astype(np.float32)

        a_aug = np.concatenate([xf_core, ones_row], axis=0)
        b_aug = np.concatenate([xf_core[:, :NI], shift_row[None, :]], axis=0)
        # pretile [N, 65] -> [128, NJT*65] so SBUF loads are contiguous chunks
        xft = (
            np.concatenate([xf_core.T, np.ones((N, 1), np.float32)], axis=1)
            .reshape(NJT, 128, C + 1)
            .transpose(1, 0, 2)
            .reshape(128, NJT * (C + 1))
        )
        import ml_dtypes

        xfb = xft.astype(ml_dtypes.bfloat16)
        in_maps.append(
            {
                "a_aug": np.ascontiguousarray(a_aug),
                "b_aug": np.ascontiguousarray(b_aug),
                "xft": np.ascontiguousarray(xft),
                "xfb": np.ascontiguousarray(xfb),
            }
        )
    return in_maps


def gather_output(results: list[dict[str, np.ndarray]]) -> np.ndarray:
    out_full = np.empty((B_, C, N), dtype=np.float32)
    for k in range(8):
        b, half = k // 2, k % 2
        i0 = half * NI
        outT = results[k]["outT"]  # [2048, 64] rows=i, cols=c
        out_full[b][:, i0 : i0 + NI] = outT.T
    return out_full.reshape(B_, C, H, W)


def kernel_run(x: np.ndarray, trace: bool = False):
    nc = build_nc()
    in_maps = prepare_in_maps(x)
    r = run_bass_kernel_spmd(nc, in_maps, list(range(8)), trace=trace)
    out = gather_output(r.results)
    return out, (r.exec_time_ns if trace else None)


def kernel(**inputs: np.ndarray) -> np.ndarray:
    out, _ = kernel_run(inputs["x"], trace=False)
    return out
